# revision 1
# baseline (speedup 1.0000x reference)
"""Trainium2 Bass kernel for Canglong shifted-window sparse attention.

Full (unsharded) inputs in, full output out. Internally shards the 960
(lon,window-type) windows across 8 NeuronCores (120 windows each), runs a
Bass/Tile kernel per core via PJRT SPMD, gathers results.

Pipeline per window (N=144 tokens, C=192, H=6 heads, d=32):
  qkv proj -> per-head S^T = k q^T (+bias+mask via DMA'd fused table) ->
  exp -> PV (E^T v) and Z = col-sums via ones-matmul -> normalize -> proj.

Layout choices (see comments inline): scores kept transposed [m,n] so the
PV matmul needs no transpose; softmax denominator via all-ones stationary
matmul; per-column normalize via reciprocal-approx + elementwise multiply.
"""
import sys

sys.path.insert(0, "/opt/trn_rl_repo")

import numpy as np

import concourse.bacc as bacc
import concourse.mybir as mybir
from concourse import tile

# Problem constants (hardcoded per harness contract)
B_ = 30          # lon window groups
NW = 32          # window types (pl x lat)
N = 144          # tokens per window
C = 192          # channels
H = 6            # heads
D = 32           # head dim
NCORES = 8
W_PER_CORE = (B_ * NW) // NCORES  # 120
TOK = N          # tokens per window
F32 = mybir.dt.float32

_CACHE = {}


def build_program(W):
    """Build + compile the per-core Bass program processing W windows."""
    key = ("v1", W)
    if key in _CACHE:
        return _CACHE[key]

    nc = bacc.Bacc("TRN2", target_bir_lowering=False, debug=False)
    T = W * TOK

    xT_d = nc.dram_tensor("xT", [193, T], F32, kind="ExternalInput")
    B_d = nc.dram_tensor("Bt", [W, 3, N, 288], F32, kind="ExternalInput")
    wqk_d = nc.dram_tensor("wqk", [193, 384], F32, kind="ExternalInput")
    wv_d = nc.dram_tensor("wv", [193, 192], F32, kind="ExternalInput")
    wp_d = nc.dram_tensor("wp", [193, 192], F32, kind="ExternalInput")
    yT_d = nc.dram_tensor("yT", [192, T], F32, kind="ExternalOutput")

    EXP = mybir.ActivationFunctionType.Exp
    ADD = mybir.AluOpType.add
    MUL = mybir.AluOpType.mult

    with tile.TileContext(nc) as tc:
        with (
            tc.tile_pool(name="const", bufs=1) as cpool,
            tc.tile_pool(name="xin", bufs=3) as xpool,
            tc.tile_pool(name="qk", bufs=2) as qkpool,
            tc.tile_pool(name="vbuf", bufs=2) as vpool,
            tc.tile_pool(name="bt", bufs=3) as bpool,
            tc.tile_pool(name="ebuf", bufs=2) as epool,
            tc.tile_pool(name="rz", bufs=2) as rzpool,
            tc.tile_pool(name="yrow", bufs=2) as ypool,
            tc.tile_pool(name="qkv_ps", bufs=2, space="PSUM") as qkv_ps,
            tc.tile_pool(name="s_ps", bufs=2, space="PSUM") as s_ps,
            tc.tile_pool(name="sb_ps", bufs=2, space="PSUM") as sb_ps,
            tc.tile_pool(name="pv_ps", bufs=1, space="PSUM") as pv_ps,
            tc.tile_pool(name="zb_ps", bufs=1, space="PSUM") as zb_ps,
        ):
            # ---- resident weights ----
            wqk0 = cpool.tile([128, 384], F32, tag="wqk0")
            wqk1 = cpool.tile([65, 384], F32, tag="wqk1")
            wv0 = cpool.tile([128, 192], F32, tag="wv0")
            wv1 = cpool.tile([65, 192], F32, tag="wv1")
            wp0 = cpool.tile([128, 192], F32, tag="wp0")
            wp1 = cpool.tile([65, 192], F32, tag="wp1")
            ones32 = cpool.tile([128, 32], F32, tag="ones32")
            nc.sync.dma_start(wqk0[:], wqk_d[0:128, :])
            nc.sync.dma_start(wqk1[:], wqk_d[128:193, :])
            nc.sync.dma_start(wv0[:], wv_d[0:128, :])
            nc.sync.dma_start(wv1[:], wv_d[128:193, :])
            nc.sync.dma_start(wp0[:], wp_d[0:128, :])
            nc.sync.dma_start(wp1[:], wp_d[128:193, :])
            nc.gpsimd.memset(ones32[:], 1.0)

            for wi in range(W):
                t0 = wi * TOK
                # ---- load x^T (plus ones row at c=192) ----
                xa = xpool.tile([128, TOK], F32, tag="xa")
                xb = xpool.tile([65, TOK], F32, tag="xb")
                nc.sync.dma_start(xa[:], xT_d[0:128, t0:t0 + TOK])
                nc.sync.dma_start(xb[:], xT_d[128:193, t0:t0 + TOK])

                # ---- qk projection: 3 chunks of 96 (qX,qY,kX,kY layout) ----
                # wqk cols: [q0q1q2 | q3q4q5 | k0k1k2 | k3k4k5] each 96
                qX = qkpool.tile([96, TOK], F32, tag="qX")
                qY = qkpool.tile([96, TOK], F32, tag="qY")
                kX = qkpool.tile([96, TOK], F32, tag="kX")
                kY = qkpool.tile([96, TOK], F32, tag="kY")
                for j, dst in enumerate((qX, qY, kX, kY)):
                    ps = qkv_ps.tile([96, TOK], F32, tag="qkv")
                    nc.tensor.matmul(ps[:], wqk0[:, 96 * j:96 * j + 96], xa[:],
                                     start=True, stop=False)
                    nc.tensor.matmul(ps[:], wqk1[:, 96 * j:96 * j + 96], xb[:],
                                     start=False, stop=True)
                    nc.scalar.copy(dst[:], ps[:])

                # ---- v projection (token-major): v[tok, 192] ----
                vA = vpool.tile([128, 192], F32, tag="vA")
                vB = vpool.tile([16, 192], F32, tag="vB")
                psv = qkv_ps.tile([128, 192], F32, tag="qkv")
                nc.tensor.matmul(psv[:], xa[:, 0:128], wv0[:], start=True, stop=False)
                nc.tensor.matmul(psv[:], xb[:, 0:128], wv1[:], start=False, stop=True)
                nc.scalar.copy(vA[:], psv[:])
                psv2 = qkv_ps.tile([16, 192], F32, tag="qkv")
                nc.tensor.matmul(psv2[:], xa[:, 128:144], wv0[:], start=True, stop=False)
                nc.tensor.matmul(psv2[:], xb[:, 128:144], wv1[:], start=False, stop=True)
                nc.scalar.copy(vB[:], psv2[:])

                # ---- per head-pair attention ----
                yA = ypool.tile([128, N], F32, tag="yA")   # proj rhs rows 0..127
                yB = ypool.tile([65, N], F32, tag="yB")    # rows 128..192 (+ones)
                for p in range(3):
                    h0, h1 = 2 * p, 2 * p + 1

                    def qslice(h, lo, sz):
                        t = qX if h < 3 else qY
                        return t[32 * (h % 3):32 * (h % 3) + 32, lo:lo + sz]

                    def kslice(h, lo, sz):
                        t = kX if h < 3 else kY
                        return t[32 * (h % 3):32 * (h % 3) + 32, lo:lo + sz]

                    # S^T [m, (h', n)] in psum, m chunks 128+16
                    sa = s_ps.tile([128, 288], F32, tag="sa")
                    sb = sb_ps.tile([16, 288], F32, tag="sb")
                    for hh, h in enumerate((h0, h1)):
                        nc.tensor.matmul(sa[:, 144 * hh:144 * hh + 144],
                                         kslice(h, 0, 128), qslice(h, 0, N),
                                         start=True, stop=True)
                        nc.tensor.matmul(sb[:, 144 * hh:144 * hh + 144],
                                         kslice(h, 128, 16), qslice(h, 0, N),
                                         start=True, stop=True)

                    # + (bias+mask) then exp
                    Ba = bpool.tile([128, 288], F32, tag="Ba")
                    Bb = bpool.tile([16, 288], F32, tag="Bb")
                    nc.sync.dma_start(Ba[:], B_d[wi, p, 0:128, :])
                    nc.sync.dma_start(Bb[:], B_d[wi, p, 128:144, :])
                    Ea = epool.tile([128, 288], F32, tag="Ea")
                    Eb = epool.tile([16, 288], F32, tag="Eb")
                    nc.vector.tensor_tensor(Ea[:], sa[:], Ba[:], ADD)
                    nc.vector.tensor_tensor(Eb[:], sb[:], Bb[:], ADD)
                    nc.scalar.activation(Ea[:], Ea[:], EXP)
                    nc.scalar.activation(Eb[:], Eb[:], EXP)

                    # PV and Z (ones-matmul) -> [64, N] psum each
                    pv = pv_ps.tile([64, N], F32, tag="pv")
                    zb = zb_ps.tile([64, N], F32, tag="zb")
                    for hh, h in enumerate((h0, h1)):
                        o = 32 * hh
                        nc.tensor.matmul(pv[o:o + 32, :], vA[:, 32 * h:32 * h + 32],
                                         Ea[:, 144 * hh:144 * hh + 144],
                                         start=True, stop=False)
                        nc.tensor.matmul(pv[o:o + 32, :], vB[:, 32 * h:32 * h + 32],
                                         Eb[:, 144 * hh:144 * hh + 144],
                                         start=False, stop=True)
                        nc.tensor.matmul(zb[o:o + 32, :], ones32[0:128, :],
                                         Ea[:, 144 * hh:144 * hh + 144],
                                         start=True, stop=False)
                        nc.tensor.matmul(zb[o:o + 32, :], ones32[0:16, :],
                                         Eb[:, 144 * hh:144 * hh + 144],
                                         start=False, stop=True)

                    # normalize: y^T rows = pv * (1/zb)
                    rz = rzpool.tile([64, N], F32, tag="rz")
                    nc.vector.reciprocal_approx_fast(rz[:], zb[:])
                    dst = yA[64 * p:64 * p + 64, :] if p < 2 else yB[0:64, :]
                    nc.vector.tensor_tensor(dst, pv[:], rz[:], MUL)

                # ones row for b_proj
                nc.gpsimd.memset(yB[64:65, :], 1.0)

                # ---- output projection ----
                pp0 = qkv_ps.tile([128, N], F32, tag="qkv")
                nc.tensor.matmul(pp0[:], wp0[:, 0:128], yA[:], start=True, stop=False)
                nc.tensor.matmul(pp0[:], wp1[:, 0:128], yB[:], start=False, stop=True)
                yo0 = ypool.tile([128, N], F32, tag="yo0")
                nc.scalar.copy(yo0[:], pp0[:])
                nc.sync.dma_start(yT_d[0:128, t0:t0 + TOK], yo0[:])
                pp1 = qkv_ps.tile([64, N], F32, tag="qkv")
                nc.tensor.matmul(pp1[:], wp0[:, 128:192], yA[:], start=True, stop=False)
                nc.tensor.matmul(pp1[:], wp1[:, 128:192], yB[:], start=False, stop=True)
                yo1 = ypool.tile([64, N], F32, tag="yo1")
                nc.scalar.copy(yo1[:], pp1[:])
                nc.sync.dma_start(yT_d[128:192, t0:t0 + TOK], yo1[:])

    nc.compile()
    _CACHE[key] = nc
    return nc


def prep_inputs(x, mask, w_qkv, b_qkv, bias_table, w_proj, b_proj, pos_index,
                ncores=NCORES):
    """Host-side prep: shard windows, build fused bias+mask tables, layouts."""
    x = np.asarray(x, np.float32)
    mask = np.asarray(mask, np.float32)
    w_qkv = np.asarray(w_qkv, np.float32)
    b_qkv = np.asarray(b_qkv, np.float32)
    bias_table = np.asarray(bias_table, np.float32)
    w_proj = np.asarray(w_proj, np.float32)
    b_proj = np.asarray(b_proj, np.float32)
    pos_index = np.asarray(pos_index)

    scale = float(D) ** -0.5
    # weights with scale folded into q part; bias as row 192 (ones-row trick)
    wq = w_qkv[:, 0:192] * scale
    wk = w_qkv[:, 192:384]
    wv = w_qkv[:, 384:576]
    bq = b_qkv[0:192] * scale
    bk = b_qkv[192:384]
    bv = b_qkv[384:576]
    # qk chunk layout: [q0q1q2 | q3q4q5 | k0k1k2 | k3k4k5]
    wqk = np.concatenate([wq[:, 0:96], wq[:, 96:192], wk[:, 0:96], wk[:, 96:192]],
                         axis=1)
    bqk = np.concatenate([bq[0:96], bq[96:192], bk[0:96], bk[96:192]])
    wqk_full = np.concatenate([wqk, bqk[None, :]], axis=0)          # [193, 384]
    wv_full = np.concatenate([wv, bv[None, :]], axis=0)             # [193, 192]
    wp_full = np.concatenate([w_proj, b_proj[None, :]], axis=0)     # [193, 192]

    # fused bias+mask, transposed: Bt[w, h, m, n] = table[idx[n,m], w, h] + mask[lon, w, n, m]
    # gather with pos_index.T so [m, n] indexing is direct
    bt = bias_table[pos_index.T.reshape(-1)].reshape(N, N, NW, H)   # [m, n, w, h]
    bt = bt.transpose(2, 3, 0, 1)                                   # [w, h, m, n]
    maskT = mask.transpose(0, 1, 3, 2)                              # [lon, w, m, n]

    nwin = B_ * NW
    Wc = nwin // ncores
    xw = x.reshape(nwin, N, C)
    in_maps = []
    for c in range(ncores):
        s = c * Wc
        wins = np.arange(s, s + Wc)
        lons, wts = wins // NW, wins % NW
        xT = np.empty((193, Wc * N), np.float32)
        xT[0:192] = xw[s:s + Wc].transpose(2, 0, 1).reshape(C, Wc * N)
        xT[192] = 1.0
        # B table: [Wc, 3, 144, 288]
        Bt = bt[wts] + maskT[lons, wts][:, None]                    # [Wc, 6, m, n]
        Bt = Bt.reshape(Wc, 3, 2, N, N).transpose(0, 1, 3, 2, 4)    # [Wc,3,m,2,n]
        Bt = np.ascontiguousarray(Bt.reshape(Wc, 3, N, 288), np.float32)
        in_maps.append({
            "xT": xT, "Bt": Bt, "wqk": wqk_full, "wv": wv_full, "wp": wp_full,
        })
    return in_maps


def gather_output(results, ncores=NCORES):
    nwin = B_ * NW
    Wc = nwin // ncores
    parts = []
    for c in range(ncores):
        yT = results[c]["yT"]                                       # [192, Wc*144]
        parts.append(yT.reshape(C, Wc, N).transpose(1, 2, 0))       # [Wc, N, C]
    y = np.concatenate(parts, axis=0)                               # [960, N, C]
    return np.ascontiguousarray(y.reshape(B_, NW, N, C), np.float32)


def kernel(x, mask, w_qkv, b_qkv, bias_table, w_proj, b_proj, pos_index):
    from concourse import bass2jax
    in_maps = prep_inputs(x, mask, w_qkv, b_qkv, bias_table, w_proj, b_proj,
                          pos_index)
    nc = build_program(W_PER_CORE)
    results = bass2jax.run_bass_via_pjrt(nc, in_maps, n_cores=NCORES)
    return gather_output(results)

